# revision 13
# baseline (speedup 1.0000x reference)
"""GraphSAGE-style GNN layer on 8 Trainium2 NeuronCores.

out = relu(W @ concat([features[nodes], mean(features[neigh_idx], 1)], 1).T)

Strategy: data-parallel over the 16384-node batch (2048 nodes/core). The
previous dma_gather kernel was bottlenecked by Q7 SWDGE descriptor
generation (~4 ns/descriptor x 22528 row-descriptors ~= 90 us); the host
prep there already materialized ~99% of the expanded rows anyway (the
windows barely dedup), so this version goes all the way: the host writes
each core's rows in exact consumption order and the device streams them
contiguously via HWDGE at HBM line rate -- no indexed DMA at all.

Per-core device pipeline:
  - one HWDGE ring streams: identity, neighbor chunks (row-major fp8e4m3,
    slot-pairs interleaved for DoubleRow), with the self rows (feature-major
    bf16) spliced in as two pieces so nothing else competes with the stream
  - PE transposes + sums the 10 neighbor rows per node by accumulating
    fp8 DoubleRow identity matmuls into PSUM (2 slots per matmul,
    alternating banks so array fills overlap drains); a short identity
    warm-up burst lifts the HAM clock gate before the real work arrives
  - PSUM->SBUF bf16 copies split across VectorE / ScalarE
  - 128x512 W-matmuls (bf16 W folded with the /10 mean scale), ScalarE
    ReLU to bf16, HWDGE store on the second ring; host casts to fp32.
"""
import numpy as np

N_CORES = 8
F = 256
E = 256
B = 16384
NSAMP = 10
P = 128
B_LOCAL = B // N_CORES          # 2048
TILES = B_LOCAL // P            # 16 transpose groups of 128 nodes
NB_COLS = TILES * NSAMP * F     # 40960 fp8 elems per partition
TCOLS = NSAMP * F               # 2560 elems per tile group
NBLK = 4                        # output column blocks
BLK = B_LOCAL // NBLK           # 512
USE_DR = True                   # fp8 DoubleRow (2 slots per matmul)
WARMUP_MM = 32                  # identity matmuls to lift the HAM clock gate

_cache = {}


def _build():
    import concourse.bacc as bacc
    import concourse.mybir as mybir
    import concourse.tile as tile

    bf16 = mybir.dt.bfloat16
    f8 = mybir.dt.float8e4
    f32 = mybir.dt.float32
    Act = mybir.ActivationFunctionType

    nc = bacc.Bacc("TRN2", target_bir_lowering=False, debug=False)
    nb = nc.dram_tensor("nb", [P, NB_COLS], f8, kind="ExternalInput")
    sb = nc.dram_tensor("sb", [P, 2 * B_LOCAL], bf16, kind="ExternalInput")
    wt = nc.dram_tensor("wt", [P, 8 * P], bf16, kind="ExternalInput")
    idt = nc.dram_tensor("idt", [P, 2 * P], f8, kind="ExternalInput")
    out = nc.dram_tensor("out", [E, B_LOCAL], bf16, kind="ExternalOutput")

    with tile.TileContext(nc) as tc:
        with (
            tc.tile_pool(name="const", bufs=1) as constp,
            tc.tile_pool(name="nbp", bufs=1) as nbp,
            tc.tile_pool(name="outs", bufs=4) as outsp,
            tc.tile_pool(name="ppA", bufs=3, space="PSUM") as ppA,
            tc.tile_pool(name="ppB", bufs=3, space="PSUM") as ppB,
            tc.tile_pool(name="pmp", bufs=1, space="PSUM") as pmp,
        ):
            # SP ring: identity first, then the neighbor stream with the
            # self rows spliced in (keeps one ring saturated, no competing
            # traffic); ACT ring: weights early, outputs late.
            idt_t = constp.tile([P, 2 * P], f8)
            nc.sync.dma_start(out=idt_t[:], in_=idt.ap())
            N = nbp.tile([P, NB_COLS], f8)
            for d in range(8):
                nc.sync.dma_start(
                    out=N[:, d * 2 * TCOLS:(d + 1) * 2 * TCOLS],
                    in_=nb.ap()[:, d * 2 * TCOLS:(d + 1) * 2 * TCOLS],
                )
            wt_t = constp.tile([P, 8 * P], bf16)
            nc.scalar.dma_start(out=wt_t[:], in_=wt.ap())
            sb_t = constp.tile([P, 2 * B_LOCAL], bf16)
            nc.scalar.dma_start(out=sb_t[:], in_=sb.ap())
            nsum = constp.tile([P, 2 * B_LOCAL], bf16)

            # warm-up burst: lifts the HAM clock gate (~3.4us of activity)
            # while the first neighbor chunk is still in flight
            wu = pmp.tile([P, P], f32, tag="pm0", name="warmup")
            for k in range(WARMUP_MM):
                nc.tensor.matmul(out=wu[:], lhsT=idt_t[:, 0:P],
                                 rhs=idt_t[:, 0:P],
                                 start=(k == 0), stop=(k == WARMUP_MM - 1))

            def emit_block(blk):
                # out[ec*128+j, blk*512+b] = relu(sum_kc wt_kc^T @ rhs_kc)
                for ec in range(2):
                    pm = pmp.tile([P, 512], f32, tag=f"pm{ec}")
                    for kc in range(4):
                        src = sb_t if kc < 2 else nsum
                        cc = kc % 2
                        nc.tensor.matmul(
                            out=pm[:],
                            lhsT=wt_t[:, kc * 256 + ec * P: kc * 256 + (ec + 1) * P],
                            rhs=src[:, cc * B_LOCAL + blk * BLK:
                                    cc * B_LOCAL + (blk + 1) * BLK],
                            start=(kc == 0), stop=(kc == 3),
                        )
                    o = outsp.tile([P, BLK], bf16, tag=f"o{ec}")
                    if ec == 0:
                        nc.vector.tensor_scalar_max(o[:], pm[:], 0.0)
                        eng = nc.sync
                    else:
                        nc.scalar.activation(o[:], pm[:], Act.Relu)
                        eng = nc.scalar
                    eng.dma_start(
                        out=out.ap()[ec * P:(ec + 1) * P,
                                     blk * BLK:(blk + 1) * BLK],
                        in_=o[:],
                    )

            for t in range(TILES):
                # transpose+sum the 10 neighbor rows of 128 nodes:
                # ptA/ptB accumulate feature chunk 0/1 in alternating banks
                ptA = ppA.tile([P, P], f32, tag="ptA")
                ptB = ppB.tile([P, P], f32, tag="ptB")
                base = t * TCOLS
                if USE_DR:
                    rhs_i = idt_t[:].rearrange("p (k b) -> p k b", k=2)
                    for sp in range(NSAMP // 2):
                        off = base + sp * 2 * F
                        for cc, pt in ((0, ptA), (1, ptB)):
                            nc.tensor.matmul(
                                out=pt[:],
                                lhsT=N[:, off + cc * 2 * P: off + (cc + 1) * 2 * P]
                                .rearrange("p (k j) -> p k j", k=2),
                                rhs=rhs_i,
                                start=(sp == 0), stop=(sp == NSAMP // 2 - 1),
                                perf_mode=mybir.MatmulPerfMode.DoubleRow,
                            )
                else:
                    for sp in range(NSAMP // 2):
                        for k in range(2):
                            off = base + sp * 2 * F + k * P
                            for cc, pt in ((0, ptA), (1, ptB)):
                                nc.tensor.matmul(
                                    out=pt[:],
                                    lhsT=N[:, off + cc * 2 * P:
                                           off + cc * 2 * P + P],
                                    rhs=idt_t[:, 0:P],
                                    start=(sp == 0 and k == 0),
                                    stop=(sp == NSAMP // 2 - 1 and k == 1),
                                )
                nc.vector.tensor_copy(out=nsum[:, t * P:(t + 1) * P],
                                      in_=ptA[:])
                nc.scalar.activation(nsum[:, B_LOCAL + t * P:
                                          B_LOCAL + (t + 1) * P],
                                     ptB[:], Act.Copy)
                # emit each output block two tiles after its nsum completes
                # so the PSUM-copy latency hides behind later transposes
                if t in (5, 9, 13):
                    emit_block((t - 5) // 4)
            emit_block(NBLK - 1)
    nc.compile()
    return nc


def _get_nc():
    if "nc" not in _cache:
        _cache["nc"] = _build()
    return _cache["nc"]


def _prep(features, W, nodes, neigh_idx):
    """Host-side layout: per-core streams in exact consumption order."""
    import ml_dtypes

    bf16 = ml_dtypes.bfloat16
    f8 = ml_dtypes.float8_e4m3
    feats = np.asarray(features, dtype=np.float32)
    W = np.asarray(W, dtype=np.float32)
    nodes = np.asarray(nodes).astype(np.int64).reshape(N_CORES, B_LOCAL)
    neigh = np.asarray(neigh_idx).astype(np.int64).reshape(
        N_CORES, B_LOCAL, NSAMP)

    feats_bf = feats.astype(bf16)
    feats_8 = feats.astype(f8)

    # wt[p, kc*256 + ec*128 + j] = Wmod[ec*128 + j, kc*128 + p]
    Wmod = np.concatenate([W[:, :F], W[:, F:] / NSAMP], axis=1)
    wtile = np.ascontiguousarray(
        Wmod.reshape(2, P, 4, P).transpose(3, 2, 0, 1).reshape(P, 8 * P)
    ).astype(bf16)

    # [I | I] so DoubleRow's k-packed moving operand replays the identity
    eye = np.eye(P, dtype=f8)
    ident = np.ascontiguousarray(np.concatenate([eye, eye], axis=1))

    in_maps = []
    for c in range(N_CORES):
        # self rows, feature-major: sb[p, cc*2048 + b] = S[b, cc*128 + p]
        S = feats_bf[nodes[c]]                       # [2048, 256]
        sbuf = np.ascontiguousarray(
            S.reshape(B_LOCAL, 2, P).transpose(2, 1, 0).reshape(P, 2 * B_LOCAL))
        # neighbor rows: nb[p, t*2560 + sp*512 + cc*256 + k*128 + j]
        #   = features[neigh[t*128+p, sp*2+k], cc*128 + j]  (fp8)
        NF = feats_8[neigh[c]]                       # [2048, 10, 256]
        nbuf = np.ascontiguousarray(
            NF.reshape(TILES, P, NSAMP // 2, 2, 2, P)
            .transpose(1, 0, 2, 4, 3, 5).reshape(P, NB_COLS))
        in_maps.append({"nb": nbuf, "sb": sbuf, "wt": wtile, "idt": ident})
    return in_maps


def run(features, W, nodes, neigh_idx, trace=False):
    from concourse.bass_utils import run_bass_kernel_spmd

    in_maps = _prep(features, W, nodes, neigh_idx)
    res = run_bass_kernel_spmd(_get_nc(), in_maps,
                               core_ids=list(range(N_CORES)), trace=trace)
    out = np.concatenate(
        [np.asarray(r["out"]).astype(np.float32) for r in res.results], axis=1)
    return out, res


def kernel(features, W, nodes, neigh_idx):
    out, _ = run(features, W, nodes, neigh_idx)
    return out


# revision 15
# speedup vs baseline: 1.0794x; 1.0794x over previous
"""GraphSAGE-style GNN layer on 8 Trainium2 NeuronCores.

out = relu(W @ concat([features[nodes], mean(features[neigh_idx], 1)], 1).T)

Strategy: data-parallel over the 16384-node batch (2048 nodes/core). The
previous dma_gather kernel was bottlenecked by Q7 SWDGE descriptor
generation (~4 ns/descriptor x 22528 row-descriptors ~= 90 us); the host
prep there already materialized ~99% of the expanded rows anyway (the
windows barely dedup), so this version goes all the way: the host writes
each core's rows in exact consumption order and the device streams them
contiguously via HWDGE at HBM line rate -- no indexed DMA at all.

Per-core device pipeline:
  - one HWDGE ring streams: identity, neighbor chunks (row-major fp8e4m3,
    slot-pairs interleaved for DoubleRow), with the self rows (feature-major
    bf16) spliced in as two pieces so nothing else competes with the stream
  - PE transposes + sums the 10 neighbor rows per node by accumulating
    fp8 DoubleRow identity matmuls into PSUM (2 slots per matmul,
    alternating banks so array fills overlap drains); a short identity
    warm-up burst lifts the HAM clock gate before the real work arrives
  - PSUM->SBUF bf16 copies split across VectorE / ScalarE
  - 128x512 W-matmuls (bf16 W folded with the /10 mean scale), ScalarE
    ReLU to bf16, HWDGE store on the second ring; host casts to fp32.
"""
import numpy as np

N_CORES = 8
F = 256
E = 256
B = 16384
NSAMP = 10
P = 128
B_LOCAL = B // N_CORES          # 2048
TILES = B_LOCAL // P            # 16 transpose groups of 128 nodes
NB_COLS = TILES * NSAMP * F     # 40960 fp8 elems per partition
TCOLS = NSAMP * F               # 2560 elems per tile group
NBLK = 4                        # output column blocks
BLK = B_LOCAL // NBLK           # 512
USE_DR = True                   # fp8 DoubleRow (2 slots per matmul)
WARMUP_MM = 32                  # identity matmuls to lift the HAM clock gate

_cache = {}


def _build():
    import concourse.bacc as bacc
    import concourse.mybir as mybir
    import concourse.tile as tile

    bf16 = mybir.dt.bfloat16
    f8 = mybir.dt.float8e4
    f32 = mybir.dt.float32
    Act = mybir.ActivationFunctionType

    nc = bacc.Bacc("TRN2", target_bir_lowering=False, debug=False)
    nb = nc.dram_tensor("nb", [P, NB_COLS], f8, kind="ExternalInput")
    sb = nc.dram_tensor("sb", [P, 2 * B_LOCAL], bf16, kind="ExternalInput")
    wt = nc.dram_tensor("wt", [P, 8 * P], bf16, kind="ExternalInput")
    idt = nc.dram_tensor("idt", [P, 2 * P], f8, kind="ExternalInput")
    out = nc.dram_tensor("out", [E, B_LOCAL], bf16, kind="ExternalOutput")

    with tile.TileContext(nc) as tc:
        with (
            tc.tile_pool(name="const", bufs=1) as constp,
            tc.tile_pool(name="nbp", bufs=1) as nbp,
            tc.tile_pool(name="outs", bufs=4) as outsp,
            tc.tile_pool(name="ppA", bufs=2, space="PSUM") as ppA,
            tc.tile_pool(name="ppB", bufs=2, space="PSUM") as ppB,
            tc.tile_pool(name="pmp", bufs=2, space="PSUM") as pmp,
        ):
            # SP ring: identity first, then the neighbor stream with the
            # self rows spliced in (keeps one ring saturated, no competing
            # traffic); ACT ring: weights early, outputs late.
            idt_t = constp.tile([P, 2 * P], f8)
            nc.sync.dma_start(out=idt_t[:], in_=idt.ap())
            N = nbp.tile([P, NB_COLS], f8)
            sb_t = constp.tile([P, 2 * B_LOCAL], bf16)
            sbv = sb_t[:].rearrange("p (c b) -> p c b", c=2)
            for d in range(8):
                nc.sync.dma_start(
                    out=N[:, d * 2 * TCOLS:(d + 1) * 2 * TCOLS],
                    in_=nb.ap()[:, d * 2 * TCOLS:(d + 1) * 2 * TCOLS],
                )
                if d in (1, 3):
                    h = d // 2  # self-rows half, spliced into the stream
                    nc.sync.dma_start(
                        out=sbv[:, :, h * 1024:(h + 1) * 1024],
                        in_=sb.ap().rearrange("p (c b) -> p c b", c=2)
                        [:, :, h * 1024:(h + 1) * 1024],
                    )
            wt_t = constp.tile([P, 8 * P], bf16)
            nc.scalar.dma_start(out=wt_t[:], in_=wt.ap())
            nsum = constp.tile([P, 2 * B_LOCAL], bf16)

            # warm-up burst: lifts the HAM clock gate (~3.4us of activity)
            # while the first neighbor chunk is still in flight
            wu = pmp.tile([P, P], f32, tag="pm0", name="warmup")
            for k in range(WARMUP_MM):
                nc.tensor.matmul(out=wu[:], lhsT=idt_t[:, 0:P],
                                 rhs=idt_t[:, 0:P],
                                 start=(k == 0), stop=(k == WARMUP_MM - 1))

            def emit_block(blk):
                # out[ec*128+j, blk*512+b] = relu(sum_kc wt_kc^T @ rhs_kc)
                for ec in range(2):
                    pm = pmp.tile([P, 512], f32, tag=f"pm{ec}")
                    for kc in range(4):
                        src = sb_t if kc < 2 else nsum
                        cc = kc % 2
                        nc.tensor.matmul(
                            out=pm[:],
                            lhsT=wt_t[:, kc * 256 + ec * P: kc * 256 + (ec + 1) * P],
                            rhs=src[:, cc * B_LOCAL + blk * BLK:
                                    cc * B_LOCAL + (blk + 1) * BLK],
                            start=(kc == 0), stop=(kc == 3),
                        )
                    o = outsp.tile([P, BLK], bf16, tag=f"o{ec}")
                    if ec == 0:
                        nc.vector.tensor_scalar_max(o[:], pm[:], 0.0)
                        eng = nc.sync
                    else:
                        nc.scalar.activation(o[:], pm[:], Act.Relu)
                        eng = nc.scalar
                    eng.dma_start(
                        out=out.ap()[ec * P:(ec + 1) * P,
                                     blk * BLK:(blk + 1) * BLK],
                        in_=o[:],
                    )

            for t in range(TILES):
                # transpose+sum the 10 neighbor rows of 128 nodes:
                # ptA/ptB accumulate feature chunk 0/1 in alternating banks
                ptA = ppA.tile([P, P], f32, tag="ptA")
                ptB = ppB.tile([P, P], f32, tag="ptB")
                base = t * TCOLS
                if USE_DR:
                    rhs_i = idt_t[:].rearrange("p (k b) -> p k b", k=2)
                    for sp in range(NSAMP // 2):
                        off = base + sp * 2 * F
                        for cc, pt in ((0, ptA), (1, ptB)):
                            nc.tensor.matmul(
                                out=pt[:],
                                lhsT=N[:, off + cc * 2 * P: off + (cc + 1) * 2 * P]
                                .rearrange("p (k j) -> p k j", k=2),
                                rhs=rhs_i,
                                start=(sp == 0), stop=(sp == NSAMP // 2 - 1),
                                perf_mode=mybir.MatmulPerfMode.DoubleRow,
                            )
                else:
                    for sp in range(NSAMP // 2):
                        for k in range(2):
                            off = base + sp * 2 * F + k * P
                            for cc, pt in ((0, ptA), (1, ptB)):
                                nc.tensor.matmul(
                                    out=pt[:],
                                    lhsT=N[:, off + cc * 2 * P:
                                           off + cc * 2 * P + P],
                                    rhs=idt_t[:, 0:P],
                                    start=(sp == 0 and k == 0),
                                    stop=(sp == NSAMP // 2 - 1 and k == 1),
                                )
                nc.vector.tensor_copy(out=nsum[:, t * P:(t + 1) * P],
                                      in_=ptA[:])
                nc.scalar.activation(nsum[:, B_LOCAL + t * P:
                                          B_LOCAL + (t + 1) * P],
                                     ptB[:], Act.Copy)
                # emit each output block two tiles after its nsum completes
                # so the PSUM-copy latency hides behind later transposes
                if t in (5, 9, 13):
                    emit_block((t - 5) // 4)
            emit_block(NBLK - 1)
    nc.compile()
    return nc


def _get_nc():
    if "nc" not in _cache:
        _cache["nc"] = _build()
    return _cache["nc"]


def _prep(features, W, nodes, neigh_idx):
    """Host-side layout: per-core streams in exact consumption order."""
    import ml_dtypes

    bf16 = ml_dtypes.bfloat16
    f8 = ml_dtypes.float8_e4m3
    feats = np.asarray(features, dtype=np.float32)
    W = np.asarray(W, dtype=np.float32)
    nodes = np.asarray(nodes).astype(np.int64).reshape(N_CORES, B_LOCAL)
    neigh = np.asarray(neigh_idx).astype(np.int64).reshape(
        N_CORES, B_LOCAL, NSAMP)

    feats_bf = feats.astype(bf16)
    feats_8 = feats.astype(f8)

    # wt[p, kc*256 + ec*128 + j] = Wmod[ec*128 + j, kc*128 + p]
    Wmod = np.concatenate([W[:, :F], W[:, F:] / NSAMP], axis=1)
    wtile = np.ascontiguousarray(
        Wmod.reshape(2, P, 4, P).transpose(3, 2, 0, 1).reshape(P, 8 * P)
    ).astype(bf16)

    # [I | I] so DoubleRow's k-packed moving operand replays the identity
    eye = np.eye(P, dtype=f8)
    ident = np.ascontiguousarray(np.concatenate([eye, eye], axis=1))

    in_maps = []
    for c in range(N_CORES):
        # self rows, feature-major: sb[p, cc*2048 + b] = S[b, cc*128 + p]
        S = feats_bf[nodes[c]]                       # [2048, 256]
        sbuf = np.ascontiguousarray(
            S.reshape(B_LOCAL, 2, P).transpose(2, 1, 0).reshape(P, 2 * B_LOCAL))
        # neighbor rows: nb[p, t*2560 + sp*512 + cc*256 + k*128 + j]
        #   = features[neigh[t*128+p, sp*2+k], cc*128 + j]  (fp8)
        NF = feats_8[neigh[c]]                       # [2048, 10, 256]
        nbuf = np.ascontiguousarray(
            NF.reshape(TILES, P, NSAMP // 2, 2, 2, P)
            .transpose(1, 0, 2, 4, 3, 5).reshape(P, NB_COLS))
        in_maps.append({"nb": nbuf, "sb": sbuf, "wt": wtile, "idt": ident})
    return in_maps


def run(features, W, nodes, neigh_idx, trace=False):
    from concourse.bass_utils import run_bass_kernel_spmd

    in_maps = _prep(features, W, nodes, neigh_idx)
    res = run_bass_kernel_spmd(_get_nc(), in_maps,
                               core_ids=list(range(N_CORES)), trace=trace)
    out = np.concatenate(
        [np.asarray(r["out"]).astype(np.float32) for r in res.results], axis=1)
    return out, res


def kernel(features, W, nodes, neigh_idx):
    out, _ = run(features, W, nodes, neigh_idx)
    return out


# revision 21
# speedup vs baseline: 1.1653x; 1.0796x over previous
"""GraphSAGE-style GNN layer on 8 Trainium2 NeuronCores.

out = relu(W @ concat([features[nodes], mean(features[neigh_idx], 1)], 1).T)

Strategy: data-parallel over the 16384-node batch (2048 nodes/core). The
previous dma_gather kernel was bottlenecked by Q7 SWDGE descriptor
generation (~4 ns/descriptor x 22528 row-descriptors ~= 90 us); the host
prep there already materialized ~99% of the expanded rows anyway (the
windows barely dedup), so this version goes all the way: the host writes
each core's rows in exact consumption order and the device streams them
contiguously via HWDGE at HBM line rate -- no indexed DMA at all.

Per-core device pipeline:
  - one HWDGE ring streams: identity, neighbor chunks (row-major fp8e4m3,
    slot-pairs interleaved for DoubleRow), with the self rows (feature-major
    bf16) spliced in as two pieces so nothing else competes with the stream
  - PE transposes + sums the 10 neighbor rows per node by accumulating
    fp8 DoubleRow identity matmuls into PSUM (2 slots per matmul,
    alternating banks so array fills overlap drains); a short identity
    warm-up burst lifts the HAM clock gate before the real work arrives
  - PSUM->SBUF bf16 copies split across VectorE / ScalarE
  - 128x512 W-matmuls (bf16 W folded with the /10 mean scale), ScalarE
    ReLU to bf16, HWDGE store on the second ring; host casts to fp32.
"""
import numpy as np

N_CORES = 8
F = 256
E = 256
B = 16384
NSAMP = 10
P = 128
B_LOCAL = B // N_CORES          # 2048
TILES = B_LOCAL // P            # 16 transpose groups of 128 nodes
NB_COLS = TILES * NSAMP * F     # 40960 fp8 elems per partition
TCOLS = NSAMP * F               # 2560 elems per tile group
NBLK = 4                        # output column blocks
BLK = B_LOCAL // NBLK           # 512
USE_DR = True                   # fp8 DoubleRow (2 slots per matmul)
USE_SWI = True                  # DoubleRowSwInterleave (pre-interleaved weights)
WARMUP_MM = 32                  # identity matmuls to lift the HAM clock gate
# neighbor chunk sizes in tiles: finer at the end so the tail transposes
# start as soon as their tile lands
CHUNK_TILES = [2, 2, 2, 2, 2, 2, 1, 1, 1, 1]

_cache = {}


def _build():
    import concourse.bacc as bacc
    import concourse.mybir as mybir
    import concourse.tile as tile

    bf16 = mybir.dt.bfloat16
    f8 = mybir.dt.float8e4
    f32 = mybir.dt.float32
    Act = mybir.ActivationFunctionType

    nc = bacc.Bacc("TRN2", target_bir_lowering=False, debug=False)
    nb = nc.dram_tensor("nb", [P, NB_COLS], f8, kind="ExternalInput")
    sb = nc.dram_tensor("sb", [P, 2 * B_LOCAL], bf16, kind="ExternalInput")
    wt = nc.dram_tensor("wt", [P, 8 * P], bf16, kind="ExternalInput")
    idt = nc.dram_tensor("idt", [P, 2 * P], f8, kind="ExternalInput")
    out = nc.dram_tensor("out", [E, B_LOCAL], bf16, kind="ExternalOutput")

    with tile.TileContext(nc) as tc:
        with (
            tc.tile_pool(name="const", bufs=1) as constp,
            tc.tile_pool(name="nbp", bufs=1) as nbp,
            tc.tile_pool(name="outs", bufs=4) as outsp,
            tc.tile_pool(name="ppA", bufs=2, space="PSUM") as ppA,
            tc.tile_pool(name="ppB", bufs=2, space="PSUM") as ppB,
            tc.tile_pool(name="pmp", bufs=2, space="PSUM") as pmp,
        ):
            # SP ring: identity first, then the neighbor stream with the
            # self rows spliced in (keeps one ring saturated, no competing
            # traffic); ACT ring: weights early, outputs late.
            idt_t = constp.tile([P, 2 * P], f8)
            nc.sync.dma_start(out=idt_t[:], in_=idt.ap())
            N = nbp.tile([P, NB_COLS], f8)
            sb_t = constp.tile([P, 2 * B_LOCAL], bf16)
            sbv = sb_t[:].rearrange("p (c b) -> p c b", c=2)
            t0 = 0
            for d, nt in enumerate(CHUNK_TILES):
                nc.sync.dma_start(
                    out=N[:, t0 * TCOLS:(t0 + nt) * TCOLS],
                    in_=nb.ap()[:, t0 * TCOLS:(t0 + nt) * TCOLS],
                )
                t0 += nt
                if d in (2, 3, 4, 5):
                    q = d - 2  # self-rows quarter, spliced into the stream
                    nc.sync.dma_start(
                        out=sbv[:, :, q * 512:(q + 1) * 512],
                        in_=sb.ap().rearrange("p (c b) -> p c b", c=2)
                        [:, :, q * 512:(q + 1) * 512],
                    )
            wt_t = constp.tile([P, 8 * P], bf16)
            nc.scalar.dma_start(out=wt_t[:], in_=wt.ap())
            nsum = constp.tile([P, 2 * B_LOCAL], bf16)

            # warm-up burst: lifts the HAM clock gate (~3.4us of activity)
            # while the first neighbor chunk is still in flight
            wu = pmp.tile([P, P], f32, tag="pm0", name="warmup")
            for k in range(WARMUP_MM):
                nc.tensor.matmul(out=wu[:], lhsT=idt_t[:, 0:P],
                                 rhs=idt_t[:, 0:P],
                                 start=(k == 0), stop=(k == WARMUP_MM - 1))

            def emit_block(blk):
                # out[ec*128+j, blk*512+b] = relu(sum_kc wt_kc^T @ rhs_kc)
                for ec in range(2):
                    pm = pmp.tile([P, 512], f32, tag=f"pm{ec}")
                    for kc in range(4):
                        src = sb_t if kc < 2 else nsum
                        cc = kc % 2
                        nc.tensor.matmul(
                            out=pm[:],
                            lhsT=wt_t[:, kc * 256 + ec * P: kc * 256 + (ec + 1) * P],
                            rhs=src[:, cc * B_LOCAL + blk * BLK:
                                    cc * B_LOCAL + (blk + 1) * BLK],
                            start=(kc == 0), stop=(kc == 3),
                        )
                    o = outsp.tile([P, BLK], bf16, tag=f"o{ec}")
                    if ec == 0:
                        nc.vector.tensor_scalar_max(o[:], pm[:], 0.0)
                        # q1 is a pure input stream until it drains; only the
                        # final block's ec0 store may ride it (stream is done)
                        eng = nc.sync if blk == NBLK - 1 else nc.scalar
                    else:
                        nc.scalar.activation(o[:], pm[:], Act.Relu)
                        eng = nc.scalar
                    eng.dma_start(
                        out=out.ap()[ec * P:(ec + 1) * P,
                                     blk * BLK:(blk + 1) * BLK],
                        in_=o[:],
                    )

            for t in range(TILES):
                # emit each output block before this tile's transposes so a
                # ready block is not head-of-line blocked behind transposes
                # that still wait on their chunk
                if t in (6, 10, 14):
                    emit_block((t - 6) // 4)
                # transpose+sum the 10 neighbor rows of 128 nodes:
                # ptA/ptB accumulate feature chunk 0/1 in alternating banks
                ptA = ppA.tile([P, P], f32, tag="ptA")
                ptB = ppB.tile([P, P], f32, tag="ptB")
                base = t * TCOLS
                if USE_DR:
                    pmode = (mybir.MatmulPerfMode.DoubleRowSwInterleave
                             if USE_SWI else mybir.MatmulPerfMode.DoubleRow)
                    rhs_i = idt_t[:].rearrange("p (k b) -> p k b", k=2)
                    for sp in range(NSAMP // 2):
                        off = base + sp * 2 * F
                        for cc, pt in ((0, ptA), (1, ptB)):
                            nc.tensor.matmul(
                                out=pt[:],
                                lhsT=N[:, off + cc * 2 * P: off + (cc + 1) * 2 * P]
                                .rearrange("p (k j) -> p k j", k=2),
                                rhs=rhs_i,
                                start=(sp == 0), stop=(sp == NSAMP // 2 - 1),
                                perf_mode=pmode,
                            )
                else:
                    for sp in range(NSAMP // 2):
                        for k in range(2):
                            off = base + sp * 2 * F + k * P
                            for cc, pt in ((0, ptA), (1, ptB)):
                                nc.tensor.matmul(
                                    out=pt[:],
                                    lhsT=N[:, off + cc * 2 * P:
                                           off + cc * 2 * P + P],
                                    rhs=idt_t[:, 0:P],
                                    start=(sp == 0 and k == 0),
                                    stop=(sp == NSAMP // 2 - 1 and k == 1),
                                )
                nc.vector.tensor_copy(out=nsum[:, t * P:(t + 1) * P],
                                      in_=ptA[:])
                nc.scalar.activation(nsum[:, B_LOCAL + t * P:
                                          B_LOCAL + (t + 1) * P],
                                     ptB[:], Act.Copy)
            emit_block(NBLK - 1)
    nc.compile()
    return nc


def _get_nc():
    if "nc" not in _cache:
        _cache["nc"] = _build()
    return _cache["nc"]


def _prep(features, W, nodes, neigh_idx):
    """Host-side layout: per-core streams in exact consumption order."""
    import ml_dtypes

    bf16 = ml_dtypes.bfloat16
    f8 = ml_dtypes.float8_e4m3
    feats = np.asarray(features, dtype=np.float32)
    W = np.asarray(W, dtype=np.float32)
    nodes = np.asarray(nodes).astype(np.int64).reshape(N_CORES, B_LOCAL)
    neigh = np.asarray(neigh_idx).astype(np.int64).reshape(
        N_CORES, B_LOCAL, NSAMP)

    feats_bf = feats.astype(bf16)
    feats_8 = feats.astype(f8)

    # wt[p, kc*256 + ec*128 + j] = Wmod[ec*128 + j, kc*128 + p]
    Wmod = np.concatenate([W[:, :F], W[:, F:] / NSAMP], axis=1)
    wtile = np.ascontiguousarray(
        Wmod.reshape(2, P, 4, P).transpose(3, 2, 0, 1).reshape(P, 8 * P)
    ).astype(bf16)

    # [I | I] so DoubleRow's k-packed moving operand replays the identity
    eye = np.eye(P, dtype=f8)
    ident = np.ascontiguousarray(np.concatenate([eye, eye], axis=1))

    in_maps = []
    for c in range(N_CORES):
        # self rows, feature-major: sb[p, cc*2048 + b] = S[b, cc*128 + p]
        S = feats_bf[nodes[c]]                       # [2048, 256]
        sbuf = np.ascontiguousarray(
            S.reshape(B_LOCAL, 2, P).transpose(2, 1, 0).reshape(P, 2 * B_LOCAL))
        # neighbor rows (fp8), one 256-col block per (t, sp, cc):
        #   DoubleRow:        [k, j]  (slot-pair member major)
        #   SwInterleave:     [A127 B127 A126 B126 ... A0 B0]
        #   (pairs element-interleaved, features reversed -- the layout the
        #   HW expects pre-interleaved in SBUF, per bass_interp.py)
        NF = feats_8[neigh[c]]                       # [2048, 10, 256]
        arr = NF.reshape(TILES, P, NSAMP // 2, 2, 2, P)   # [t,p,sp,k,cc,j]
        if USE_SWI:
            nbuf = np.ascontiguousarray(
                arr.transpose(1, 0, 2, 4, 5, 3)[:, :, :, :, ::-1, :]
                .reshape(P, NB_COLS))
        else:
            nbuf = np.ascontiguousarray(
                arr.transpose(1, 0, 2, 4, 3, 5).reshape(P, NB_COLS))
        in_maps.append({"nb": nbuf, "sb": sbuf, "wt": wtile, "idt": ident})
    return in_maps


def run(features, W, nodes, neigh_idx, trace=False):
    from concourse.bass_utils import run_bass_kernel_spmd

    in_maps = _prep(features, W, nodes, neigh_idx)
    res = run_bass_kernel_spmd(_get_nc(), in_maps,
                               core_ids=list(range(N_CORES)), trace=trace)
    out = np.concatenate(
        [np.asarray(r["out"]).astype(np.float32) for r in res.results], axis=1)
    return out, res


def kernel(features, W, nodes, neigh_idx):
    out, _ = run(features, W, nodes, neigh_idx)
    return out
